# revision 2
# baseline (speedup 1.0000x reference)
"""MLA kernel for Trainium2, 8-core SPMD — v3 (packed-DMA restructure).

Sharding: 2 batch groups x 4-way tensor parallel within group.
  core r: batch g = r // 4, rank-in-group rg = r % 4,
          stage-1 tokens  [g, rg*256:(rg+1)*256],
          heads           [rg*8:(rg+1)*8],
          output d-rows   [rg*1024:(rg+1)*1024] (ReduceScatter slices).

Pipeline per core:
  stage 1: fusedT = wfa^T @ hiddenT for 256 tokens in 3 column passes
           (kv+rope first -> AG1a over 4 ranks; then q in 2 passes -> AG1b)
  phase B: per 512-token block: k/v up-proj (overlaps AG1b), q up-proj,
           causal attention with diagonal free-dim shrinking,
           local out-projection (full D, contraction over own 8 heads)
           -> per-block ReduceScatter(add) over the 4-group, the last
           block split in two d-halves to shrink the exposed tail.

All large DRAM operands are host-packed so each matrix loads with a
single DMA of multi-KB per-partition lines (HWDGE descriptor count is
the stage-1 bottleneck otherwise). All data planes bf16; PSUM fp32.
"""

import numpy as np

import concourse.bass as bass
import concourse.bacc as bacc
import concourse.mybir as mybir
import concourse.tile as tile
from concourse.bass_utils import run_bass_kernel_spmd

F32 = mybir.dt.float32
F32R = mybir.dt.float32r
BF16 = mybir.dt.bfloat16
AF = mybir.ActivationFunctionType

# ---- model dims ----
B, S, D = 2, 1024, 4096
H, QL, KVL = 32, 1536, 512
NOPE, ROPE, VD = 128, 64, 128
QKD = NOPE + ROPE
ROPE_BASE = 10000.0
EPS = 1e-6
NC = 8
GS = 4

P = 128
TPC = S // GS               # 256
HPC = H // GS               # 8
DPC = D // GS               # 1024
RH = ROPE // 2              # 32
TBLK = 512
NT = S // TBLK              # 2
KB = TBLK // P              # 4
NREG = TBLK // TPC          # 2
NKV = KVL // P              # 4
NQ = QL // P                # 12
DCN = D // P                # 32
SCALE = 1.0 / float(np.sqrt(np.float32(QKD)))
KVR = KVL + ROPE            # 576

# stage-1 segments of the (host-permuted [kv|rope|q]) wfa columns.
# Each segment is one DMA; chunks are 128 wide except rope (64).
# (name, pass, [chunk widths])
SEGS = [
    ("kv0", 0, [P, P]),
    ("kv1", 0, [P, P, ROPE]),
    ("qa0", 1, [P, P, P]),
    ("qa1", 1, [P, P, P]),
    ("qb0", 2, [P, P, P]),
    ("qb1", 2, [P, P, P]),
]
SEGW = [sum(ws) for (_, _, ws) in SEGS]
SEGOFF = np.concatenate([[0], np.cumsum([32 * w for w in SEGW])]).tolist()
WFA_COLS = SEGOFF[-1]  # 32 * 2112


def _rope_rotate(nc, pool, out64, in64, cos_sb, sin_sb, width, tag):
    """NeoX half-rotation on a [64, width] tile (in64 may be PSUM)."""
    half = 32
    x1 = pool.tile([half, width], BF16, name=f"{tag}_x1", tag="rope_x", bufs=2)
    x2 = pool.tile([half, width], BF16, name=f"{tag}_x2", tag="rope_x", bufs=2)
    nc.vector.tensor_copy(x1[:], in64[0:half, :])
    nc.vector.tensor_copy(x2[:], in64[half:2 * half, :])
    t1 = pool.tile([half, width], BF16, name=f"{tag}_t1", tag="rope_t", bufs=2)
    t2 = pool.tile([half, width], BF16, name=f"{tag}_t2", tag="rope_t", bufs=2)
    nc.vector.tensor_mul(t1[:], x1[:], cos_sb)
    nc.vector.tensor_mul(t2[:], x2[:], sin_sb)
    nc.vector.tensor_sub(out64[0:half, :], t1[:], t2[:])
    t3 = pool.tile([half, width], BF16, name=f"{tag}_t3", tag="rope_t", bufs=2)
    t4 = pool.tile([half, width], BF16, name=f"{tag}_t4", tag="rope_t", bufs=2)
    nc.vector.tensor_mul(t3[:], x1[:], sin_sb)
    nc.vector.tensor_mul(t4[:], x2[:], cos_sb)
    nc.vector.tensor_add(out64[half:2 * half, :], t3[:], t4[:])


def build_program(cfg=None, reps=1, tiny_out=False, no_cc=False):
    nc = bacc.Bacc("TRN2", target_bir_lowering=False, debug=False,
                   num_devices=NC)

    # ---- kernel I/O (host-packed per-core views) ----
    ht_d = nc.dram_tensor("htp", [P, DCN * TPC], BF16, kind="ExternalInput")
    wfa_d = nc.dram_tensor("wfap", [P, WFA_COLS], BF16, kind="ExternalInput")
    wqn_d = nc.dram_tensor("wqnp", [P, NQ * HPC * NOPE], BF16,
                           kind="ExternalInput")
    wqr_d = nc.dram_tensor("wqrp", [P, NQ * HPC * ROPE], BF16,
                           kind="ExternalInput")
    wkn_d = nc.dram_tensor("wknp", [P, NKV * HPC * NOPE], BF16,
                           kind="ExternalInput")
    wv_d = nc.dram_tensor("wvp", [P, NKV * HPC * VD], BF16,
                          kind="ExternalInput")
    wo_d = nc.dram_tensor("wop", [P, HPC * D], BF16, kind="ExternalInput")
    cosq_d = nc.dram_tensor("cosq", [RH, S], BF16, kind="ExternalInput")
    sinq_d = nc.dram_tensor("sinq", [RH, S], BF16, kind="ExternalInput")
    cosk_d = nc.dram_tensor("cosk", [RH, TPC], BF16, kind="ExternalInput")
    sink_d = nc.dram_tensor("sink", [RH, TPC], BF16, kind="ExternalInput")
    tri_d = nc.dram_tensor("tri", [P, P], BF16, kind="ExternalInput")
    onesb_d = nc.dram_tensor("onesb", [P, P], BF16, kind="ExternalInput")
    onesf_d = nc.dram_tensor("onesf", [P, P], F32, kind="ExternalInput")

    # RS pieces: (tau, d_lo, d_hi)
    PIECES = [(0, 0, D), (1, 0, D // 2), (1, D // 2, D)]
    if tiny_out:
        out_blk = {(t, lo, hi): nc.dram_tensor(
            f"op{t}_{lo}_scratch", [(hi - lo) // GS, TBLK], BF16)
            for (t, lo, hi) in PIECES}
        tick_d = nc.dram_tensor("tick", [P, 4], BF16, kind="ExternalOutput")
    else:
        out_blk = {(t, lo, hi): nc.dram_tensor(
            f"op{t}_{lo}", [(hi - lo) // GS, TBLK], BF16,
            kind="ExternalOutput") for (t, lo, hi) in PIECES}
        tick_d = None
    rs_out_t = {(t, lo, hi): nc.dram_tensor(
        f"rso{t}_{lo}", [(hi - lo) // GS, TBLK], BF16)
        for (t, lo, hi) in PIECES}

    RG = [[0, 1, 2, 3], [4, 5, 6, 7]]

    with tile.TileContext(nc) as tc:
        with (
            tc.tile_pool(name="const", bufs=1) as cst,
            tc.tile_pool(name="dram", bufs=1, space="DRAM") as drp,
        ):
            # ---------- tiny constants ----------
            ones_b = cst.tile([P, P], BF16, name="ones_b")
            nc.sync.dma_start(ones_b[:], onesb_d[:, :])
            ones_fr = cst.tile([P, P], F32R, name="ones_fr")
            nc.sync.dma_start(ones_fr[:], onesf_d[:, :].bitcast(F32R))
            tri_b = cst.tile([P, P], BF16, name="tri_b")
            nc.scalar.dma_start(tri_b[:], tri_d[:, :])
            eps_t = cst.tile([P, 1], F32, name="eps_t")
            nc.vector.memset(eps_t[:], EPS)
            cosq_sb = cst.tile([RH, S], BF16, name="cosq")
            sinq_sb = cst.tile([RH, S], BF16, name="sinq")
            cosk_sb = cst.tile([RH, TPC], BF16, name="cosk")
            sink_sb = cst.tile([RH, TPC], BF16, name="sink")
            nc.scalar.dma_start(cosq_sb[:], cosq_d[:, :])
            nc.scalar.dma_start(sinq_sb[:], sinq_d[:, :])
            nc.scalar.dma_start(cosk_sb[:], cosk_d[:, :])
            nc.scalar.dma_start(sink_sb[:], sink_d[:, :])

            # ---------- packed weight tiles (one DMA each, staged) ----------
            wqn_all = cst.tile([P, NQ * HPC * NOPE], BF16, name="wqn_all")
            wqr_all = cst.tile([P, NQ * HPC * ROPE], BF16, name="wqr_all")
            wkn_all = cst.tile([P, NKV * HPC * NOPE], BF16, name="wkn_all")
            wv_all = cst.tile([P, NKV * HPC * VD], BF16, name="wv_all")
            wo_all = cst.tile([P, HPC * D], BF16, name="wo_all")

            def wqn(cc, a, b):
                return wqn_all[:, cc * HPC * NOPE + a:cc * HPC * NOPE + b]

            def wqr(cc, a, b):
                return wqr_all[:, cc * HPC * ROPE + a:cc * HPC * ROPE + b]

            def wkn(cc, a, b):
                return wkn_all[:, cc * HPC * NOPE + a:cc * HPC * NOPE + b]

            def wv(cc, a, b):
                return wv_all[:, cc * HPC * VD + a:cc * HPC * VD + b]

            def wo(hv, a, b):
                return wo_all[:, hv * D + a:hv * D + b]

            def one_rep(rep):
                agkv_in = drp.tile([KVR, TPC], BF16, name="agkv_in")
                agkv_out = drp.tile([GS, KVR, TPC], BF16, name="agkv_out")
                agq_in = drp.tile([QL, TPC], BF16, name="agq_in")
                agq_out = drp.tile([GS, QL, TPC], BF16, name="agq_out")
                rs_in = [drp.tile([D, TBLK], BF16, name=f"rs_in{t}")
                         for t in range(NT)]

                def rs_piece(tau, lo, hi):
                    if not no_cc:
                        nc.gpsimd.collective_compute(
                            "ReduceScatter", mybir.AluOpType.add,
                            replica_groups=RG,
                            ins=[rs_in[tau][lo:hi, :]],
                            outs=[rs_out_t[(tau, lo, hi)][:, :]])
                    nc.sync.dma_start(out_blk[(tau, lo, hi)][:, :],
                                      rs_out_t[(tau, lo, hi)][:, :])

                # ---------- stage 1 ----------
                with (
                    tc.tile_pool(name="s1", bufs=1) as s1,
                    tc.tile_pool(name="s1ps", bufs=1, space="PSUM") as s1ps,
                ):
                    ht_all = s1.tile([P, DCN * TPC], BF16, name="ht_all")
                    for q_ in range(4):
                        w = DCN * TPC // 4
                        nc.sync.dma_start(ht_all[:, q_ * w:(q_ + 1) * w],
                                          ht_d[:, q_ * w:(q_ + 1) * w])

                    def ht(dc):
                        return ht_all[:, dc * TPC:(dc + 1) * TPC]

                    ps_sq = s1ps.tile([P, TPC], F32, name="ps_sumq",
                                      tag="sums", bufs=2)
                    ps_skv = s1ps.tile([P, TPC], F32, name="ps_sumkv",
                                       tag="sums", bufs=2)
                    psf = {}        # live chunk psums: (segname, j) -> ps
                    fused_q = {}    # q chunk stash (bf16) for the fb mul
                    nq_seen = 0

                    def emit_seg(si):
                        nonlocal nq_seen
                        (nm, _, ws) = SEGS[si]
                        segw = SEGW[si]
                        wt = s1.tile([P, 32 * 384], BF16, name=f"wfa_{nm}",
                                     tag="wfa", bufs=2)
                        nc.sync.dma_start(
                            wt[:, :32 * segw],
                            wfa_d[:, SEGOFF[si]:SEGOFF[si] + 32 * segw])
                        for j, cw in enumerate(ws):
                            psf[(nm, j)] = s1ps.tile(
                                [P, TPC], F32, name=f"psf_{nm}_{j}",
                                tag="fch", bufs=6)
                        for dc in range(DCN):
                            off = dc * segw
                            for j, cw in enumerate(ws):
                                nc.tensor.matmul(
                                    psf[(nm, j)][:cw, :],
                                    wt[:, off:off + cw], ht(dc),
                                    start=(dc == 0), stop=(dc == DCN - 1))
                                off += cw
                        # bf16 stash + squares into the rmsnorm accumulators
                        for j, cw in enumerate(ws):
                            if cw == ROPE:
                                continue  # rope chunk: no norm
                            ft = s1.tile([P, TPC], BF16, name=f"fs_{nm}_{j}",
                                         tag="fs", bufs=NQ + NKV)
                            nc.scalar.copy(ft[:], psf[(nm, j)][:, :])
                            fused_q[(nm, j)] = ft
                            del psf[(nm, j)]
                            x2 = s1.tile([P, TPC], F32R, name=f"x2_{nm}_{j}",
                                         tag="x2", bufs=4)
                            nc.vector.tensor_mul(x2[:], ft[:], ft[:])
                            if si < 2:
                                kvi = si * 2 + j
                                nc.tensor.matmul(
                                    ps_skv[:, :], ones_fr[:], x2[:],
                                    start=(kvi == 0), stop=(kvi == NKV - 1))
                            else:
                                nc.tensor.matmul(
                                    ps_sq[:, :], ones_fr[:], x2[:],
                                    start=(nq_seen == 0),
                                    stop=(nq_seen == NQ - 1))
                                nq_seen += 1

                    # -- kv pass --
                    emit_seg(0)
                    emit_seg(1)
                    rkv = s1.tile([P, TPC], F32, name="rkv")
                    sq_t = s1.tile([P, TPC], F32, name="sq_kv")
                    nc.scalar.activation(sq_t[:], ps_skv[:, :], AF.Sqrt,
                                         bias=eps_t[:], scale=1.0 / KVL)
                    nc.vector.reciprocal(rkv[:], sq_t[:])
                    for kvi in range(NKV):
                        nm, j = SEGS[kvi // 2][0], kvi % 2
                        fb = s1.tile([P, TPC], BF16, name=f"fb{kvi}", tag="fb",
                                     bufs=3)
                        nc.vector.tensor_mul(fb[:], fused_q[(nm, j)][:],
                                             rkv[:])
                        nc.sync.dma_start(agkv_in[kvi * P:(kvi + 1) * P, :],
                                          fb[:])
                    kpe_ro = s1.tile([ROPE, TPC], BF16, name="kpe_ro")
                    _rope_rotate(nc, s1, kpe_ro[:], psf[("kv1", 2)][:ROPE, :],
                                 cosk_sb[:], sink_sb[:], TPC, "kpe")
                    del psf[("kv1", 2)]
                    nc.sync.dma_start(agkv_in[KVL:KVL + ROPE, :], kpe_ro[:])
                    if not no_cc:
                        nc.gpsimd.collective_compute(
                            "AllGather", mybir.AluOpType.bypass,
                            replica_groups=RG,
                            ins=[agkv_in.opt()], outs=[agkv_out.opt()])
                    # k/v weights stream on the scalar queue meanwhile
                    if rep == 0:
                        nc.scalar.dma_start(wkn_all[:], wkn_d[:, :])
                        nc.scalar.dma_start(wv_all[:], wv_d[:, :])
                    # -- q passes --
                    for si in range(2, 6):
                        emit_seg(si)
                    rq = s1.tile([P, TPC], F32, name="rq")
                    sq_t2 = s1.tile([P, TPC], F32, name="sq_q")
                    nc.scalar.activation(sq_t2[:], ps_sq[:, :], AF.Sqrt,
                                         bias=eps_t[:], scale=1.0 / QL)
                    nc.vector.reciprocal(rq[:], sq_t2[:])
                    qi = 0
                    for si in range(2, 6):
                        (nm, _, ws) = SEGS[si]
                        for j in range(len(ws)):
                            fb = s1.tile([P, TPC], BF16, name=f"fbq{qi}",
                                         tag="fb", bufs=3)
                            nc.vector.tensor_mul(fb[:], fused_q[(nm, j)][:],
                                                 rq[:])
                            nc.sync.dma_start(agq_in[qi * P:(qi + 1) * P, :],
                                              fb[:])
                            qi += 1
                    if not no_cc:
                        nc.gpsimd.collective_compute(
                            "AllGather", mybir.AluOpType.bypass,
                            replica_groups=RG,
                            ins=[agq_in.opt()], outs=[agq_out.opt()])
                    if rep == 0:
                        nc.scalar.dma_start(wqn_all[:], wqn_d[:, :])
                        nc.scalar.dma_start(wqr_all[:], wqr_d[:, :])
                        nc.scalar.dma_start(wo_all[:], wo_d[:, :])

                # ---------- phase B + out-proj + RS ----------
                with (
                    tc.tile_pool(name="pb", bufs=1) as pb,
                    tc.tile_pool(name="pbps", bufs=1, space="PSUM") as pbps,
                ):
                    kc_t = {}
                    kpe_t = {}
                    v_t = {}

                    def kv_side(tau):
                        kvn = []
                        for cc in range(NKV):
                            t = pb.tile([P, TBLK], BF16, name=f"kvn{tau}_{cc}",
                                        tag="kvn", bufs=NKV + 1)
                            nc.sync.dma_start(
                                t[:, :].rearrange("p (r c) -> p r c", r=NREG),
                                agkv_out[tau * NREG:(tau + 1) * NREG,
                                         cc * P:(cc + 1) * P,
                                         :].transpose([1, 0, 2]))
                            kvn.append(t)
                        kp = pb.tile([ROPE, TBLK], BF16, name=f"kpe{tau}",
                                     tag="kpt", bufs=NT)
                        nc.sync.dma_start(
                            kp[:, :].rearrange("p (r c) -> p r c", r=NREG),
                            agkv_out[tau * NREG:(tau + 1) * NREG,
                                     KVL:KVL + ROPE, :].transpose([1, 0, 2]))
                        kpe_t[tau] = kp
                        for h in range(HPC):
                            ps = pbps.tile([P, TBLK], F32, name=f"psk{tau}_{h}",
                                           tag="acc", bufs=4)
                            for cc in range(NKV):
                                nc.tensor.matmul(
                                    ps[:, :], wkn(cc, h * P, (h + 1) * P),
                                    kvn[cc][:], start=(cc == 0),
                                    stop=(cc == NKV - 1))
                            kt = pb.tile([P, TBLK], BF16, name=f"kc{tau}_{h}",
                                         tag="kc", bufs=NT * HPC)
                            nc.scalar.copy(kt[:], ps[:, :])
                            kc_t[(tau, h)] = kt
                        for i in range(KB):
                            vt = pb.tile([P, HPC * VD], BF16,
                                         name=f"vt{tau}_{i}", tag="vt",
                                         bufs=NT * KB)
                            for half in range(2):
                                ps = pbps.tile([P, TBLK], F32,
                                               name=f"psv{tau}_{i}_{half}",
                                               tag="acc", bufs=4)
                                for cc in range(NKV):
                                    nc.tensor.matmul(
                                        ps[:, :],
                                        kvn[cc][:, i * P:(i + 1) * P],
                                        wv(cc, half * TBLK,
                                           (half + 1) * TBLK),
                                        start=(cc == 0),
                                        stop=(cc == NKV - 1))
                                nc.scalar.copy(
                                    vt[:, half * TBLK:(half + 1) * TBLK],
                                    ps[:, :])
                            v_t[(tau, i)] = vt

                    def q_side_attn(tau):
                        qcn = []
                        for cc in range(NQ):
                            t = pb.tile([P, TBLK], BF16, name=f"qcn{tau}_{cc}",
                                        tag="qcn", bufs=NQ)
                            nc.sync.dma_start(
                                t[:, :].rearrange("p (r c) -> p r c", r=NREG),
                                agq_out[tau * NREG:(tau + 1) * NREG,
                                        cc * P:(cc + 1) * P,
                                        :].transpose([1, 0, 2]))
                            qcn.append(t)
                        cos_sl = cosq_sb[:, tau * TBLK:(tau + 1) * TBLK]
                        sin_sl = sinq_sb[:, tau * TBLK:(tau + 1) * TBLK]
                        qtn = {}
                        qtr = {}
                        cxs = []
                        nkc = (tau + 1) * KB
                        for oc in range(HPC // 2):
                            ps = pbps.tile([P, TBLK], F32,
                                           name=f"psr{tau}_{oc}",
                                           tag="acc", bufs=4)
                            for cc in range(NQ):
                                nc.tensor.matmul(
                                    ps[:, :],
                                    wqr(cc, oc * P, (oc + 1) * P),
                                    qcn[cc][:], start=(cc == 0),
                                    stop=(cc == NQ - 1))
                            for s_ in range(2):
                                h_ = oc * 2 + s_
                                qr = pb.tile([ROPE, TBLK], BF16,
                                             name=f"qtr{tau}_{h_}",
                                             tag="qtr", bufs=HPC)
                                _rope_rotate(nc, pb, qr[:],
                                             ps[s_ * ROPE:(s_ + 1) * ROPE, :],
                                             cos_sl, sin_sl, TBLK,
                                             f"qr{tau}_{h_}")
                                qtr[h_] = qr
                        for h in range(HPC):
                            ps = pbps.tile([P, TBLK], F32, name=f"psq{tau}_{h}",
                                           tag="acc", bufs=4)
                            for cc in range(NQ):
                                nc.tensor.matmul(
                                    ps[:, :], wqn(cc, h * P, (h + 1) * P),
                                    qcn[cc][:], start=(cc == 0),
                                    stop=(cc == NQ - 1))
                            qt = pb.tile([P, TBLK], BF16, name=f"qtn{tau}_{h}",
                                         tag="qtn", bufs=3)
                            nc.vector.tensor_copy(qt[:], ps[:, :])
                            qtn[h] = qt
                            ps_den = pbps.tile([P, TBLK], F32,
                                               name=f"psd{tau}_{h}",
                                               tag="acc", bufs=4)
                            ps_ctx = pbps.tile([P, TBLK], F32,
                                               name=f"psc{tau}_{h}",
                                               tag="acc", bufs=4)
                            def acc_chunk(kc, qoff, tau_k, ik, ex):
                                nc.tensor.matmul(ps_den[:, qoff:], ones_b[:],
                                                 ex[:, qoff:],
                                                 start=(kc == 0),
                                                 stop=(kc == nkc - 1))
                                nc.tensor.matmul(
                                    ps_ctx[:, qoff:],
                                    v_t[(tau_k, ik)][:, h * P:(h + 1) * P],
                                    ex[:, qoff:], start=(kc == 0),
                                    stop=(kc == nkc - 1))

                            pend = None
                            for kc in range(nkc):
                                tau_k, ik = kc // KB, kc % KB
                                diag = (tau_k == tau)
                                qoff = ik * P if diag else 0
                                ps_s = pbps.tile([P, TBLK], F32,
                                                 name=f"pss{tau}_{h}_{kc}",
                                                 tag="tr", bufs=2)
                                nc.tensor.matmul(
                                    ps_s[:, qoff:],
                                    kc_t[(tau_k, h)][:, ik * P:(ik + 1) * P],
                                    qtn[h][:, qoff:], start=True, stop=False)
                                nc.tensor.matmul(
                                    ps_s[:, qoff:],
                                    kpe_t[tau_k][:, ik * P:(ik + 1) * P],
                                    qtr[h][:, qoff:], start=False, stop=True)
                                ex = pb.tile([P, TBLK], BF16,
                                             name=f"ex{tau}_{h}_{kc}",
                                             tag="ex", bufs=2)
                                nc.scalar.activation(ex[:, qoff:],
                                                     ps_s[:, qoff:], AF.Exp,
                                                     scale=SCALE)
                                if diag:
                                    nc.gpsimd.tensor_mul(
                                        ex[:, qoff:qoff + P],
                                        ex[:, qoff:qoff + P], tri_b[:])
                                if pend is not None:
                                    acc_chunk(*pend)
                                pend = (kc, qoff, tau_k, ik, ex)
                            acc_chunk(*pend)
                            rec = pb.tile([P, TBLK], F32, name=f"rec{tau}_{h}",
                                          tag="rec", bufs=1)
                            nc.vector.reciprocal(rec[:], ps_den[:, :])
                            cx = pb.tile([P, TBLK], BF16, name=f"cx{tau}_{h}",
                                         tag="cx", bufs=HPC)
                            nc.vector.tensor_mul(cx[:], ps_ctx[:, :], rec[:])
                            cxs.append(cx)
                        return cxs

                    def out_proj(tau, cxs):
                        pieces = [p for p in PIECES if p[0] == tau]
                        for e in range(DCN):
                            ps = pbps.tile([P, TBLK], F32, name=f"pso{tau}_{e}",
                                           tag="co", bufs=2)
                            for hv in range(HPC):
                                nc.tensor.matmul(
                                    ps[:, :], wo(hv, e * P, (e + 1) * P),
                                    cxs[hv][:], start=(hv == 0),
                                    stop=(hv == HPC - 1))
                            ot = pb.tile([P, TBLK], BF16, name=f"ot{tau}_{e}",
                                         tag="ot", bufs=2)
                            nc.scalar.copy(ot[:], ps[:, :])
                            nc.sync.dma_start(
                                rs_in[tau][e * P:(e + 1) * P, :], ot[:])
                            for (t_, lo, hi) in pieces:
                                if (e + 1) * P == hi:
                                    rs_piece(t_, lo, hi)

                    kv_side(0)
                    kv_side(1)
                    for tau in range(NT):
                        cxs = q_side_attn(tau)
                        out_proj(tau, cxs)
                    kc_t.clear()
                    kpe_t.clear()
                    v_t.clear()

            for _rep in range(reps):
                one_rep(_rep)
                if tiny_out:
                    tk = cst.tile([P, 4], BF16, name="tick_sb", tag="tick",
                                  bufs=2)
                    for jj, pc in enumerate(PIECES + [PIECES[0]]):
                        nc.sync.dma_start(
                            tk[:, jj:jj + 1],
                            out_blk[pc][0:P, jj * P:jj * P + 1])
                    nc.sync.dma_start(tick_d[:, :], tk[:])

    nc.compile()
    return nc


# ---------------- host wrapper ----------------

def _pack(m, ncols):
    """[N*128, ncols] -> [128, N*ncols]: row-chunk-major per partition."""
    n = m.shape[0] // P
    return np.ascontiguousarray(
        m.reshape(n, P, ncols).transpose(1, 0, 2).reshape(P, n * ncols))


def _host_prep(inputs, cfg=None):
    import ml_dtypes
    bf16 = ml_dtypes.bfloat16
    hs = np.asarray(inputs["hidden_states"], np.float32)
    wfa = np.asarray(inputs["w_fused_a"], np.float32)
    gq = np.asarray(inputs["q_a_ln_w"], np.float32)
    gkv = np.asarray(inputs["kv_a_ln_w"], np.float32)
    wqb = np.asarray(inputs["w_q_b"], np.float32)
    wkvb = np.asarray(inputs["w_kv_b"], np.float32)
    wo = np.asarray(inputs["w_o"], np.float32)

    # wfa permuted to [kv | rope | q], then packed per segment
    wfa_p = np.concatenate([wfa[:, QL:QL + KVL], wfa[:, QL + KVL:],
                            wfa[:, :QL]], axis=1)
    segs = []
    c0 = 0
    for si in range(len(SEGS)):
        w = SEGW[si]
        segs.append(_pack(wfa_p[:, c0:c0 + w], w))      # [128, 32*w]
        c0 += w
    wfa_packed = np.concatenate(segs, axis=1).astype(bf16)

    wq = (gq[:, None] * wqb).reshape(QL, H, QKD)
    wkv = (gkv[:, None] * wkvb).reshape(KVL, H, NOPE + VD)

    half = ROPE // 2
    inv_freq = (1.0 / (np.float32(ROPE_BASE) **
                       (np.arange(half, dtype=np.float32) / np.float32(half))))
    ang = (np.arange(S, dtype=np.float32)[:, None] * inv_freq[None, :])
    cosT = np.ascontiguousarray(np.cos(ang).astype(np.float32).T)
    sinT = np.ascontiguousarray(np.sin(ang).astype(np.float32).T)

    ii = np.arange(P)[:, None]
    jj = np.arange(P)[None, :]
    tri = (ii <= jj).astype(np.float32)

    in_maps = []
    for r in range(NC):
        g, rg = r // GS, r % GS
        tok = slice(rg * TPC, (rg + 1) * TPC)
        hd = slice(rg * HPC, (rg + 1) * HPC)
        hT = np.ascontiguousarray(hs[g, tok].T)          # [4096, 256]
        in_maps.append({
            "htp": _pack(hT, TPC).astype(bf16),
            "wfap": wfa_packed,
            "wqnp": _pack(np.ascontiguousarray(
                wq[:, hd, :NOPE].reshape(QL, HPC * NOPE)),
                HPC * NOPE).astype(bf16),
            "wqrp": _pack(np.ascontiguousarray(
                wq[:, hd, NOPE:].reshape(QL, HPC * ROPE)),
                HPC * ROPE).astype(bf16),
            "wknp": _pack(np.ascontiguousarray(
                wkv[:, hd, :NOPE].reshape(KVL, HPC * NOPE)),
                HPC * NOPE).astype(bf16),
            "wvp": _pack(np.ascontiguousarray(
                wkv[:, hd, NOPE:].reshape(KVL, HPC * VD)),
                HPC * VD).astype(bf16),
            "wop": _pack(np.ascontiguousarray(
                wo[rg * HPC * VD:(rg + 1) * HPC * VD, :]), D).astype(bf16),
            "cosq": cosT.astype(bf16),
            "sinq": sinT.astype(bf16),
            "cosk": np.ascontiguousarray(
                cosT[:, rg * TPC:(rg + 1) * TPC]).astype(bf16),
            "sink": np.ascontiguousarray(
                sinT[:, rg * TPC:(rg + 1) * TPC]).astype(bf16),
            "tri": tri.astype(bf16),
            "onesb": np.ones((P, P), bf16),
            "onesf": np.ones((P, P), np.float32),
        })
    return in_maps


def _assemble(results, cfg=None):
    out = np.zeros((B, S, D), np.float32)
    pieces = [(0, 0, D), (1, 0, D // 2), (1, D // 2, D)]
    for r in range(NC):
        g, rg = r // GS, r % GS
        for (t, lo, hi) in pieces:
            blk = np.asarray(results[r][f"op{t}_{lo}"], np.float32)
            w = (hi - lo) // GS
            d0 = lo + rg * w
            out[g, t * TBLK:(t + 1) * TBLK, d0:d0 + w] = blk.T
    return out


def kernel(**inputs):
    nc = build_program()
    in_maps = _host_prep(inputs)
    res = run_bass_kernel_spmd(nc, in_maps, list(range(NC)))
    return _assemble(res.results)
